# revision 40
# baseline (speedup 1.0000x reference)
"""Trainium2 Bass kernel for nn_AttentionBasedMerger.

Reference computation (per batch element b, SQ=1):
  q = input @ Wq + bq                      -> (NH, HD)  [tiny]
  k = retrieval @ Wk + bk                  -> (SK, NH, HD)
  v = retrieval @ Wv + bv                  -> (SK, NH, HD)
  scores[h,j] = cos_sim(q[h], k[j,h])
  p = (scores+1)/2 ; 2-way gumbel-softmax gate with external uniform noise
  probs[h,j] = gate[...,0]
  ctx[h] = sum_j probs[h,j] v[j,h]         -> (NH, HD)
  out = ctx.flat @ Wd + bd                 -> (HID,)

Measured bottleneck of the whole pipeline is host->device input bytes, so the
kernel is organised around shipping the retrieval tensor exactly ONCE, in
bf16, natural [SK, HID] layout (32MB/core), and deriving every other layout
on-device:
  - k-projection needs x^T (contraction dim on partitions): each 128x128
    subtile is transposed on the PE (bf16 transpose, 1 cyc/row).
  - m-matmul (probs^T @ x) uses the natural tiles directly as the stationary
    operand, which also produces m already transposed for the ctx GEMM.

Algebraic restructuring (exact up to fp reassociation):
  - v-projection is never materialized:
      ctx[h] = (sum_j probs[h,j] x[j]) @ Wv_h (+ (sum_j probs[h,j]) * bv_h)
  - scores come from the k-projection run once:
      s_raw[j,(b,h)] = x[j] @ swblk        (16 extra psum columns;
                                            swblk = Wk @ qhat_blockdiag,
                                            computed on host: q-projection +
                                            normalize is 0.05% of FLOPs)
      ||k||^2        = sum_d k^2           (squared-eviction + reduce)
      scores         = s_raw * rsqrt(||k||^2)
  - the 2-way gumbel softmax collapses to a stable rational that only needs
    the noise-log ratio:
      probs = p / (p + (1-p)*r),  r = A0/A1,  A_i = EPS - log(u_i + EPS)
    (r computed on host, shipped as one clamped fp16 tensor; its fp16
    relative error gives Delta probs = probs(1-probs)*dr/r <= 5e-4.)

Numerics: bf16 operands everywhere with f32 PSUM accumulation; emulated
end-to-end rel err 4.1e-3 vs the 2e-2 gate.

Sharding: pure data-parallel over batch, 8 batch elements per core.
bq is folded into the host-side qhat; the device bias path (bk/bv/bd) exists
but is not exercised by the reference (all biases are zero).
"""

import sys

sys.path.insert(0, "/opt/trn_rl_repo")

import ml_dtypes
import numpy as np

import concourse.bass as bass  # noqa: F401  (import keeps bass registered)
import concourse.tile as tile
from concourse import bacc, mybir
from concourse.bass_utils import run_bass_kernel_spmd
from concourse.masks import make_identity

F32 = mybir.dt.float32
BF16 = mybir.dt.bfloat16
F16 = mybir.dt.float16
AX = mybir.AxisListType
OP = mybir.AluOpType
AF = mybir.ActivationFunctionType

B, SK, HID, NH, HD = 64, 2048, 1024, 16, 64
NCORES = 8
BL = B // NCORES  # 8 batch elems per core
CI = HID // 128  # 8 contraction chunks
JC = SK // 128  # 16 seq chunks
EPS = 1e-20


def build_nc(nobias=True, gather_w=True):
    nc = bacc.Bacc("TRN2", target_bir_lowering=False, debug=False, num_devices=NCORES)

    def din(name, shape, dt):
        return nc.dram_tensor(name, list(shape), dt, kind="ExternalInput").ap()

    x = din("x", [BL, SK, HID], BF16)
    # the gumbel gate only needs the noise-log RATIO r = A0/A1 (see gate
    # below); shipped fp16, pre-tiled on host to [b, j%128, jc, h] for
    # contiguous 512B DMA partition lines
    rg = din("rg", [BL, 128, JC, NH], F16)
    if gather_w:
        # per-core shard of [Wk, Wv, Wd]: core c ships rows c*128..(c+1)*128
        # of each; the full weights are assembled on-device with an AllGather
        # so the replicated 6MB crosses the host link only once
        wshard = din("wshard", [3, 128, HID], BF16)
    else:
        wk_d = din("wk", [HID, HID], BF16)
        wv_d = din("wv", [HID, HID], BF16)
        wd_d = din("wd", [HID, HID], BF16)
    SW = NH if nobias else 2 * NH
    # bias builds pack the wbk norm-correction columns into swblk (cols
    # NH:SW, same for every b) so the kproj extra columns stay a SINGLE psum
    # accumulation group -- two interleaved groups in one psum tile corrupt
    # the accumulation
    swblk_d = din("swblk", [128, CI, BL, SW], BF16)
    if not nobias:
        cqn_d = din("cqn", [BL, NH], F32)
        sbrep_d = din("sbrep", [1, NH * JC], F32)
        bv_d = din("bv", [1, HID], F32)
        bd_d = din("bd", [1, HID], F32)
    out = nc.dram_tensor("o", [BL, HID], F32, kind="ExternalOutput").ap()

    with tile.TileContext(nc) as tc:
        with (
            tc.tile_pool(name="const", bufs=1) as constp,
            tc.tile_pool(name="wkp", bufs=1) as wkp,
            tc.tile_pool(name="bigw", bufs=2) as bigwp,
            tc.tile_pool(name="xn", bufs=26) as xnp,
            tc.tile_pool(name="xt", bufs=4) as xtp,
            tc.tile_pool(name="ksq", bufs=4) as ksqp,
            tc.tile_pool(name="gate", bufs=2) as gatep,
            tc.tile_pool(name="noise", bufs=2) as ap_pool,
            tc.tile_pool(name="probsp", bufs=3) as probsp,
            tc.tile_pool(name="dram", bufs=1, space="DRAM") as dramp,
            tc.tile_pool(name="pbig", bufs=2, space="PSUM") as pp,
            tc.tile_pool(name="ptrp", bufs=2, space="PSUM") as ptrp,
            tc.tile_pool(name="psml", bufs=2 if nobias else 1, space="PSUM") as pps,
        ):
            identB = constp.tile([128, 128], BF16, tag="identB")
            make_identity(nc, identB[:])

            if gather_w:
                # weight allgather: ExternalInput -> dram bounce -> AllGather.
                # wk gathers alone so the k-projection can start ~3x sooner;
                # wv/wd (epilogue-only) gather in the background.
                wbounce_k = dramp.tile([128, HID], BF16, tag="wbounce_k")
                nc.sync.dma_start(wbounce_k[:], wshard[0])
                wfull_k = dramp.tile(
                    [NCORES, 128, HID], BF16, tag="wfull_k", addr_space="Shared"
                )
                nc.gpsimd.collective_compute(
                    "AllGather",
                    OP.bypass,
                    replica_groups=[list(range(NCORES))],
                    ins=[wbounce_k[:]],
                    outs=[wfull_k[:]],
                )
                wbounce_vd = dramp.tile([2, 128, HID], BF16, tag="wbounce_vd")
                nc.sync.dma_start(wbounce_vd[:], wshard[1:3])
                wfull_vd = dramp.tile(
                    [NCORES, 2, 128, HID], BF16, tag="wfull_vd", addr_space="Shared"
                )
                nc.gpsimd.collective_compute(
                    "AllGather",
                    OP.bypass,
                    replica_groups=[list(range(NCORES))],
                    ins=[wbounce_vd[:]],
                    outs=[wfull_vd[:]],
                )
                wk_src = wfull_k.rearrange("ci p f -> p ci f")
                wv_src = wfull_vd[:, 0, :, :].rearrange("ci p f -> p ci f")
                wd_src = wfull_vd[:, 1, :, :].rearrange("ci p f -> p ci f")
            else:
                wk_src = wk_d.rearrange("(ci p) f -> p ci f", p=128)
                wv_src = wv_d.rearrange("(ci p) f -> p ci f", p=128)
                wd_src = wd_d.rearrange("(ci p) f -> p ci f", p=128)

            wk_sb = wkp.tile([128, CI, HID], BF16, tag="wk")
            nc.sync.dma_start(wk_sb[:], wk_src)
            swblk = constp.tile([128, CI, BL, SW], BF16, tag="swblk")
            nc.sync.dma_start(swblk[:], swblk_d)
            # epilogue weights: chunked loads so the FIFO DMA queue can
            # interleave xn-tile refills between them (one 5.8us monolithic
            # load starves the PE of x tiles -- 11us gap in the sim trace)
            wv_sb = bigwp.tile([128, CI, HID], BF16, tag="bigw", name="wv_sb")
            wd_sb = bigwp.tile([128, CI, HID], BF16, tag="bigw", name="wd_sb")
            if gather_w:
                for ci in range(CI):
                    nc.sync.dma_start(wv_sb[:, ci, :], wfull_vd[ci, 0, :, :])
                    nc.sync.dma_start(wd_sb[:, ci, :], wfull_vd[ci, 1, :, :])
            else:
                nc.sync.dma_start(wv_sb[:], wv_src)
                nc.sync.dma_start(wd_sb[:], wd_src)

            if not nobias:
                sb_sb = constp.tile([128, NH * JC], F32, tag="sb_sb")
                nc.sync.dma_start(sb_sb[:], sbrep_d.to_broadcast((128, NH * JC)))
                bv8 = constp.tile([BL, HID], F32, tag="bv8")
                nc.sync.dma_start(bv8[:], bv_d.to_broadcast((BL, HID)))
                bd8 = constp.tile([BL, HID], F32, tag="bd8")
                nc.sync.dma_start(bd8[:], bd_d.to_broadcast((BL, HID)))
                cqn_bc = []
                for b in range(BL):
                    t = constp.tile([128, NH], F32, tag=f"cqn{b}", name=f"cqn{b}")
                    nc.sync.dma_start(
                        t[:], cqn_d[b : b + 1, :].to_broadcast((128, NH))
                    )
                    cqn_bc.append(t)
                ones16 = constp.tile([128, 1], BF16, tag="ones16")
                nc.vector.memset(ones16[:], 1.0)
                psp = pps.tile([128, BL], F32, tag="psp")

            # m output, already transposed: mT[c, ci, b*NH+h]
            mT = constp.tile([128, CI, BL * NH], BF16, tag="mT")

            # ---------------- main loop: per local batch ----------------
            for b in range(BL):
                ssq_all = gatep.tile([128, JC, NH], F32, tag="ssq", name="ssq_all")
                sk_all = gatep.tile([128, JC, SW], F32, tag="sk", name="sk_all")
                xn_tiles = []
                for jc in range(JC):
                    xnt = xnp.tile([128, HID], BF16, tag="xn", name="xnt")
                    nc.sync.dma_start(xnt[:], x[b, jc * 128 : (jc + 1) * 128, :])
                    xn_tiles.append(xnt)
                    # on-device transpose of the 8 subtiles -> x^T for kproj
                    # (all 8 land in one psum bank; one DVE eviction)
                    xtt = xtp.tile([128, CI, 128], BF16, tag="xt", name="xtt")
                    ptr = ptrp.tile([128, CI * 128], BF16, tag="ptr", name="ptr")
                    for cc in range(CI):
                        nc.tensor.transpose(
                            ptr[:, cc * 128 : (cc + 1) * 128],
                            xnt[:, cc * 128 : (cc + 1) * 128],
                            identB[:],
                        )
                    nc.vector.tensor_copy(
                        xtt[:], ptr[:].rearrange("p (ci j) -> p ci j", j=128)
                    )
                    # k-projection + score columns
                    pk = pp.tile([128, HID], F32, tag="pk", name="pk")
                    ps = pps.tile([128, SW], F32, tag="sml", name="ps")
                    for ci in range(CI):
                        st = ci == 0
                        sp_ = ci == CI - 1
                        lhs = xtt[:, ci, :]
                        for bank in range(2):
                            fs = slice(bank * 512, (bank + 1) * 512)
                            nc.tensor.matmul(
                                pk[:, fs], lhs, wk_sb[:, ci, fs], start=st, stop=sp_
                            )
                        nc.tensor.matmul(
                            ps[:, 0:SW], lhs, swblk[:, ci, b, :], start=st, stop=sp_
                        )
                    # evictions: k^2 via ACT square; segmented reduce on DVE
                    ksq = ksqp.tile([128, HID], F32, tag="ksq", name="ksq")
                    nc.scalar.activation(ksq[:], pk[:, :], AF.Square)
                    nc.vector.reduce_sum(
                        ssq_all[:, jc, :],
                        ksq[:].rearrange("p (h d) -> p h d", d=HD),
                        axis=AX.X,
                    )
                    nc.vector.tensor_copy(sk_all[:, jc, :], ps[:, 0:SW])

                # ---------------- gate (rational gumbel softmax) ----------
                # probs = p*A1/(p*A1 + (1-p)*A0) = p / (p + (1-p)*r), r=A0/A1
                rg_t = ap_pool.tile([128, JC, NH], F16, tag="rg", name="rg_t")
                nc.sync.dma_start(rg_t[:], rg[b])
                rgf = ap_pool.tile([128, JC, NH], F32, tag="rgf", name="rgf")
                nc.vector.tensor_copy(rgf[:], rg_t[:])

                g1 = gatep.tile([128, JC, NH], F32, tag="g1", name="g1")
                g2 = gatep.tile([128, JC, NH], F32, tag="g2", name="g2")
                g3 = gatep.tile([128, JC, NH], F32, tag="g3", name="g3")
                if nobias:
                    gden = ssq_all  # ||k||^2 needs no bias correction
                else:
                    nc.vector.scalar_tensor_tensor(
                        g1[:], sk_all[:, :, NH:SW], 2.0, ssq_all[:], OP.mult, OP.add
                    )
                    nc.vector.tensor_add(
                        g1[:], g1[:], sb_sb[:].rearrange("p (jc h) -> p jc h", h=NH)
                    )
                    gden = g1
                # g2 = rsqrt(gden) with one Newton step
                nc.scalar.activation(g2[:], gden[:], AF.Sqrt)
                nc.vector.reciprocal(g2[:], g2[:])
                nc.vector.tensor_mul(g3[:], g2[:], g2[:])
                nc.vector.tensor_mul(g3[:], g3[:], gden[:])
                nc.vector.tensor_scalar(g3[:], g3[:], -0.5, 1.5, OP.mult, OP.add)
                nc.vector.tensor_mul(g2[:], g2[:], g3[:])
                # g3 = scores = (s_raw (+ cqn)) * rsqrt
                if nobias:
                    nc.vector.tensor_mul(g3[:], sk_all[:, :, 0:NH], g2[:])
                else:
                    nc.vector.tensor_add(
                        g3[:],
                        sk_all[:, :, 0:NH],
                        cqn_bc[b][:].unsqueeze(1).to_broadcast([128, JC, NH]),
                    )
                    nc.vector.tensor_mul(g3[:], g3[:], g2[:])
                # p = (scores+1)/2 ; den = p + (1-p)*r
                nc.vector.tensor_scalar(g2[:], g3[:], 0.5, 0.5, OP.mult, OP.add)
                nc.vector.tensor_scalar(g1[:], g3[:], -0.5, 0.5, OP.mult, OP.add)
                nc.vector.tensor_mul(g1[:], g1[:], rgf[:])
                nc.vector.tensor_add(g1[:], g1[:], g2[:])  # den
                # probs = num * refined_recip(den)
                nc.vector.reciprocal(g3[:], g1[:])
                nc.vector.tensor_mul(g1[:], g1[:], g3[:])
                nc.vector.tensor_scalar(g1[:], g1[:], -1.0, 2.0, OP.mult, OP.add)
                nc.vector.tensor_mul(g3[:], g3[:], g1[:])
                nc.vector.tensor_mul(g2[:], g2[:], g3[:])  # probs (f32)
                probs = probsp.tile([128, JC, NH], BF16, tag="probs", name="probs")
                nc.vector.tensor_copy(probs[:], g2[:])

                # ---------------- m-matmul: m^T[c, h] = sum_j x[j,c] probs[j,h]
                # natural xn tiles as stationary -> output lands pre-transposed
                for cc in range(CI):
                    pm = pps.tile([128, NH], F32, tag="sml", name="pm")
                    for jc in range(JC):
                        nc.tensor.matmul(
                            pm[:, :],
                            xn_tiles[jc][:, cc * 128 : (cc + 1) * 128],
                            probs[:, jc, :],
                            start=(jc == 0),
                            stop=(jc == JC - 1),
                        )
                    nc.vector.tensor_copy(mT[:, cc, b * NH : (b + 1) * NH], pm[:, :])
                if not nobias:
                    # sp[h, b] = sum_j probs
                    for jc in range(JC):
                        nc.tensor.matmul(
                            psp[0:NH, b : b + 1],
                            probs[:, jc, :],
                            ones16[:],
                            start=(jc == 0),
                            stop=(jc == JC - 1),
                        )

            # ---------------- ctx + final dense ------------------------------
            # ctx[b, (h,d)] = sum_ci mT[:, ci, (b,h)] @ Wv[ci, (h,d)]
            pctx = pp.tile([128, HID], F32, tag="pk", name="pctx")
            for h in range(NH):
                for ci in range(CI):
                    nc.tensor.matmul(
                        pctx[0:BL, h * HD : (h + 1) * HD],
                        mT[:, ci, h : BL * NH : NH],
                        wv_sb[:, ci, h * HD : (h + 1) * HD],
                        start=(ci == 0),
                        stop=(ci == CI - 1),
                    )
            ctx_sb = constp.tile([BL, HID], BF16, tag="ctx_sb")
            if nobias:
                nc.vector.tensor_copy(ctx_sb[:], pctx[0:BL, :])
            else:
                # sp: psum [NH, BL] -> transpose -> [BL, NH]
                spT = constp.tile([NH, BL], BF16, tag="spT")
                nc.vector.tensor_copy(spT[:], psp[0:NH, 0:BL])
                ptr_sp = ptrp.tile([128, 128], BF16, tag="ptr", name="ptr_sp")
                nc.tensor.transpose(
                    ptr_sp[0:BL, 0:NH], spT[:], identB[0:NH, 0:NH]
                )
                sp_all = constp.tile([BL, NH], F32, tag="sp_all")
                nc.vector.tensor_copy(sp_all[:], ptr_sp[0:BL, 0:NH])
                ctxf = constp.tile([BL, HID], F32, tag="ctxf")
                nc.vector.tensor_mul(
                    ctxf[:].rearrange("b (h d) -> b h d", d=HD),
                    bv8[:].rearrange("b (h d) -> b h d", d=HD),
                    sp_all[:].unsqueeze(2).to_broadcast([BL, NH, HD]),
                )
                nc.vector.tensor_add(ctxf[:], ctxf[:], pctx[0:BL, :])
                nc.vector.tensor_copy(ctx_sb[:], ctxf[:])
            # transpose ctx -> [c, b]
            ctxT = constp.tile([128, CI, BL], BF16, tag="ctxT")
            for ci in range(CI):
                ptr_c = ptrp.tile([128, 128], BF16, tag="ptr", name="ptr_c")
                nc.tensor.transpose(
                    ptr_c[:, 0:BL],
                    ctx_sb[:, ci * 128 : (ci + 1) * 128],
                    identB[0:BL, 0:BL],
                )
                nc.vector.tensor_copy(ctxT[:, ci, :], ptr_c[:, 0:BL])
            # out = ctx @ Wd (+ bd)
            po = pp.tile([128, HID], F32, tag="pk", name="po")
            for ci in range(CI):
                st = ci == 0
                sp_ = ci == CI - 1
                for bank in range(2):
                    fs = slice(bank * 512, (bank + 1) * 512)
                    nc.tensor.matmul(
                        po[0:BL, fs], ctxT[:, ci, :], wd_sb[:, ci, fs],
                        start=st, stop=sp_,
                    )
            o_sb = constp.tile([BL, HID], F32, tag="o_sb")
            if nobias:
                nc.vector.tensor_copy(o_sb[:], po[0:BL, :])
            else:
                nc.vector.tensor_add(o_sb[:], po[0:BL, :], bd8[:])
            nc.sync.dma_start(out[:], o_sb[:])

    nc.compile()
    # the compiled module graph is large and permanent: freeze it out of
    # cyclic-GC scans so gen-2 collections during the steady-state dispatch
    # loop don't add tail latency
    import gc

    gc.collect()
    gc.freeze()
    return nc


def prep_in_maps(inputs, nobias=True, gather_w=True):
    """Host-side staging: batch shard, bf16/fp16 quantization, q-projection
    and gate-noise logs (both tiny compared to the device GEMMs)."""
    it = np.asarray(inputs["input_tensor"], dtype=np.float32)  # (B, 1, HID)
    rt = np.asarray(inputs["retrieval_tensor"], dtype=np.float32)  # (B, SK, HID)
    un = np.asarray(inputs["u_noise"], dtype=np.float32)  # (B, NH, 1, SK, 2)
    Wq = np.asarray(inputs["Wq"], dtype=np.float32)
    Wk = np.asarray(inputs["Wk"], dtype=np.float32)
    Wv = np.asarray(inputs["Wv"], dtype=np.float32)
    Wd = np.asarray(inputs["Wd"], dtype=np.float32)
    bq = np.asarray(inputs["bq"], dtype=np.float32).reshape(HID)

    # qhat on host (f32, matches reference precision); bq folded in here
    q = it[:, 0, :] @ Wq + bq
    qh = q.reshape(B, NH, HD)
    qh = qh / np.linalg.norm(qh, axis=-1, keepdims=True)  # (B, NH, HD)

    # swblk[c, (b,h)] = Wk @ qhat_blockdiag, laid out [p, ci, b, h]; bias
    # builds append the wbk norm-correction columns (same for every b)
    Wkh = Wk.reshape(HID, NH, HD)
    wq_eff = np.einsum("chd,bhd->cbh", Wkh, qh, optimize=True)  # (HID, B, NH)
    if not nobias:
        bk = np.asarray(inputs["bk"], dtype=np.float32).reshape(NH, HD)
        wbk = np.einsum("chd,hd->ch", Wkh, bk)  # (HID, NH)
        wq_eff = np.concatenate(
            [wq_eff, np.broadcast_to(wbk[:, None, :], wq_eff.shape)], axis=2
        )  # (HID, B, 2*NH)
    sw = wq_eff.shape[2]
    swblk = np.ascontiguousarray(
        wq_eff.reshape(CI, 128, B, sw).transpose(1, 0, 2, 3)
    ).astype(ml_dtypes.bfloat16)  # (128, CI, B, SW)

    # gate noise ratio r = A0/A1, A_i = EPS - log(u_i + EPS). Only the ratio
    # enters probs = p/(p + (1-p)r), and its fp16 relative error stays bounded
    # (Delta probs = probs(1-probs) * dr/r). Clamped into fp16's finite
    # normal range; both saturation limits give the correct probs limit.
    u0 = un[:, :, 0, :, 0].transpose(0, 2, 1)  # (B, SK, NH)
    u1 = un[:, :, 0, :, 1].transpose(0, 2, 1)
    a0 = np.float32(EPS) - np.log(u0 + np.float32(EPS), dtype=np.float32)
    a1 = np.float32(EPS) - np.log(u1 + np.float32(EPS), dtype=np.float32)
    rg = np.clip(a0 / a1, np.float32(6.2e-5), np.float32(60000.0)).astype(
        np.float16
    )
    # pre-tile to [b, j%128, jc, h] for contiguous DMA partition lines
    rg = np.ascontiguousarray(rg.reshape(B, JC, 128, NH).transpose(0, 2, 1, 3))

    x_bf = rt.astype(ml_dtypes.bfloat16)  # (B, SK, HID)

    # stacked [Wk, Wv, Wd] for the per-core weight shards
    wkvd = np.stack([Wk, Wv, Wd]).astype(ml_dtypes.bfloat16)  # (3, HID, HID)

    shared = {}
    if not gather_w:
        shared.update(wk=wkvd[0], wv=wkvd[1], wd=wkvd[2])
    if not nobias:
        shared["sbrep"] = (
            np.tile((bk**2).sum(axis=1), JC).reshape(1, NH * JC).astype(np.float32)
        )
        shared["bv"] = np.asarray(inputs["bv"], dtype=np.float32).reshape(1, HID)
        shared["bd"] = np.asarray(inputs["bd"], dtype=np.float32).reshape(1, HID)
        cqn = np.einsum("bhd,hd->bh", qh, bk).astype(np.float32)  # (B, NH)

    in_maps = []
    for c in range(NCORES):
        bs = slice(c * BL, (c + 1) * BL)
        m = {
            "x": np.ascontiguousarray(x_bf[bs]),
            "rg": np.ascontiguousarray(rg[bs]),
            "swblk": np.ascontiguousarray(swblk[:, :, bs, :]),
            **shared,
        }
        if gather_w:
            m["wshard"] = np.ascontiguousarray(wkvd[:, c * 128 : (c + 1) * 128, :])
        if not nobias:
            m["cqn"] = np.ascontiguousarray(cqn[bs])
        in_maps.append(m)
    return in_maps


_NC_CACHE = {}


def kernel(**inputs) -> np.ndarray:
    # bq is folded into the host-side q-projection, so only bk/bv/bd need the
    # device bias path
    nobias = all(
        not np.any(np.asarray(inputs[k])) for k in ("bk", "bv", "bd")
    )
    if nobias not in _NC_CACHE:
        _NC_CACHE[nobias] = build_nc(nobias)
    nc = _NC_CACHE[nobias]
    in_maps = prep_in_maps(inputs, nobias)
    res = run_bass_kernel_spmd(nc, in_maps, core_ids=list(range(NCORES)))
    return np.concatenate([res.results[c]["o"] for c in range(NCORES)], axis=0)


# revision 41
# speedup vs baseline: 1.0001x; 1.0001x over previous
"""Trainium2 Bass kernel for nn_AttentionBasedMerger.

Reference computation (per batch element b, SQ=1):
  q = input @ Wq + bq                      -> (NH, HD)  [tiny]
  k = retrieval @ Wk + bk                  -> (SK, NH, HD)
  v = retrieval @ Wv + bv                  -> (SK, NH, HD)
  scores[h,j] = cos_sim(q[h], k[j,h])
  p = (scores+1)/2 ; 2-way gumbel-softmax gate with external uniform noise
  probs[h,j] = gate[...,0]
  ctx[h] = sum_j probs[h,j] v[j,h]         -> (NH, HD)
  out = ctx.flat @ Wd + bd                 -> (HID,)

Measured bottleneck of the whole pipeline is host->device input bytes, so the
kernel is organised around shipping the retrieval tensor exactly ONCE, in
bf16, natural [SK, HID] layout (32MB/core), and deriving every other layout
on-device:
  - k-projection needs x^T (contraction dim on partitions): each 128x128
    subtile is transposed on the PE (bf16 transpose, 1 cyc/row).
  - m-matmul (probs^T @ x) uses the natural tiles directly as the stationary
    operand, which also produces m already transposed for the ctx GEMM.

Algebraic restructuring (exact up to fp reassociation):
  - v-projection is never materialized:
      ctx[h] = (sum_j probs[h,j] x[j]) @ Wv_h (+ (sum_j probs[h,j]) * bv_h)
  - scores come from the k-projection run once:
      s_raw[j,(b,h)] = x[j] @ swblk        (16 extra psum columns;
                                            swblk = Wk @ qhat_blockdiag,
                                            computed on host: q-projection +
                                            normalize is 0.05% of FLOPs)
      ||k||^2        = sum_d k^2           (squared-eviction + reduce)
      scores         = s_raw * rsqrt(||k||^2)
  - the 2-way gumbel softmax collapses to a stable rational that only needs
    the noise-log ratio:
      probs = p / (p + (1-p)*r),  r = A0/A1,  A_i = EPS - log(u_i + EPS)
    (r computed on host, shipped as one clamped fp16 tensor; its fp16
    relative error gives Delta probs = probs(1-probs)*dr/r <= 5e-4.)

Numerics: bf16 operands everywhere with f32 PSUM accumulation; emulated
end-to-end rel err 4.1e-3 vs the 2e-2 gate.

Sharding: pure data-parallel over batch, 8 batch elements per core.
bq is folded into the host-side qhat; the device bias path (bk/bv/bd) exists
but is not exercised by the reference (all biases are zero).
"""

import sys

sys.path.insert(0, "/opt/trn_rl_repo")

import ml_dtypes
import numpy as np

import concourse.bass as bass  # noqa: F401  (import keeps bass registered)
import concourse.tile as tile
from concourse import bacc, mybir
from concourse.bass_utils import run_bass_kernel_spmd
from concourse.masks import make_identity

F32 = mybir.dt.float32
BF16 = mybir.dt.bfloat16
F16 = mybir.dt.float16
AX = mybir.AxisListType
OP = mybir.AluOpType
AF = mybir.ActivationFunctionType

B, SK, HID, NH, HD = 64, 2048, 1024, 16, 64
NCORES = 8
BL = B // NCORES  # 8 batch elems per core
CI = HID // 128  # 8 contraction chunks
JC = SK // 128  # 16 seq chunks
EPS = 1e-20


def build_nc(nobias=True, gather_w=True):
    nc = bacc.Bacc("TRN2", target_bir_lowering=False, debug=False, num_devices=NCORES)

    def din(name, shape, dt):
        return nc.dram_tensor(name, list(shape), dt, kind="ExternalInput").ap()

    x = din("x", [BL, SK, HID], BF16)
    # the gumbel gate only needs the noise-log RATIO r = A0/A1 (see gate
    # below); shipped fp16, pre-tiled on host to [b, j%128, jc, h] for
    # contiguous 512B DMA partition lines
    rg = din("rg", [BL, 128, JC, NH], F16)
    if gather_w:
        # per-core shard of [Wk, Wv, Wd]: core c ships rows c*128..(c+1)*128
        # of each; the full weights are assembled on-device with an AllGather
        # so the replicated 6MB crosses the host link only once
        wshard = din("wshard", [3, 128, HID], BF16)
    else:
        wk_d = din("wk", [HID, HID], BF16)
        wv_d = din("wv", [HID, HID], BF16)
        wd_d = din("wd", [HID, HID], BF16)
    SW = NH if nobias else 2 * NH
    # bias builds pack the wbk norm-correction columns into swblk (cols
    # NH:SW, same for every b) so the kproj extra columns stay a SINGLE psum
    # accumulation group -- two interleaved groups in one psum tile corrupt
    # the accumulation
    swblk_d = din("swblk", [128, CI, BL, SW], BF16)
    if not nobias:
        cqn_d = din("cqn", [BL, NH], F32)
        sbrep_d = din("sbrep", [1, NH * JC], F32)
        bv_d = din("bv", [1, HID], F32)
        bd_d = din("bd", [1, HID], F32)
    out = nc.dram_tensor("o", [BL, HID], F32, kind="ExternalOutput").ap()

    with tile.TileContext(nc) as tc:
        with (
            tc.tile_pool(name="const", bufs=1) as constp,
            tc.tile_pool(name="wkp", bufs=1) as wkp,
            tc.tile_pool(name="bigw", bufs=2) as bigwp,
            tc.tile_pool(name="xn", bufs=26) as xnp,
            tc.tile_pool(name="xt", bufs=4) as xtp,
            tc.tile_pool(name="ksq", bufs=4) as ksqp,
            tc.tile_pool(name="gate", bufs=2) as gatep,
            tc.tile_pool(name="noise", bufs=2) as ap_pool,
            tc.tile_pool(name="probsp", bufs=3) as probsp,
            tc.tile_pool(name="dram", bufs=1, space="DRAM") as dramp,
            tc.tile_pool(name="pbig", bufs=2, space="PSUM") as pp,
            tc.tile_pool(name="ptrp", bufs=2, space="PSUM") as ptrp,
            tc.tile_pool(name="psml", bufs=2 if nobias else 1, space="PSUM") as pps,
        ):
            identB = constp.tile([128, 128], BF16, tag="identB")
            make_identity(nc, identB[:])

            if gather_w:
                # weight allgather: ExternalInput -> dram bounce -> AllGather.
                # wk gathers alone so the k-projection can start ~3x sooner;
                # wv/wd (epilogue-only) gather in the background.
                wbounce_k = dramp.tile([128, HID], BF16, tag="wbounce_k")
                nc.sync.dma_start(wbounce_k[:], wshard[0])
                wfull_k = dramp.tile(
                    [NCORES, 128, HID], BF16, tag="wfull_k", addr_space="Shared"
                )
                nc.gpsimd.collective_compute(
                    "AllGather",
                    OP.bypass,
                    replica_groups=[list(range(NCORES))],
                    ins=[wbounce_k[:]],
                    outs=[wfull_k[:]],
                )
                wbounce_vd = dramp.tile([2, 128, HID], BF16, tag="wbounce_vd")
                nc.sync.dma_start(wbounce_vd[:], wshard[1:3])
                wfull_vd = dramp.tile(
                    [NCORES, 2, 128, HID], BF16, tag="wfull_vd", addr_space="Shared"
                )
                nc.gpsimd.collective_compute(
                    "AllGather",
                    OP.bypass,
                    replica_groups=[list(range(NCORES))],
                    ins=[wbounce_vd[:]],
                    outs=[wfull_vd[:]],
                )
                wk_src = wfull_k.rearrange("ci p f -> p ci f")
                wv_src = wfull_vd[:, 0, :, :].rearrange("ci p f -> p ci f")
                wd_src = wfull_vd[:, 1, :, :].rearrange("ci p f -> p ci f")
            else:
                wk_src = wk_d.rearrange("(ci p) f -> p ci f", p=128)
                wv_src = wv_d.rearrange("(ci p) f -> p ci f", p=128)
                wd_src = wd_d.rearrange("(ci p) f -> p ci f", p=128)

            wk_sb = wkp.tile([128, CI, HID], BF16, tag="wk")
            nc.sync.dma_start(wk_sb[:], wk_src)
            swblk = constp.tile([128, CI, BL, SW], BF16, tag="swblk")
            nc.sync.dma_start(swblk[:], swblk_d)
            # epilogue weights: queue the DMA early so it rides spare bandwidth
            wv_sb = bigwp.tile([128, CI, HID], BF16, tag="bigw", name="wv_sb")
            nc.sync.dma_start(wv_sb[:], wv_src)
            wd_sb = bigwp.tile([128, CI, HID], BF16, tag="bigw", name="wd_sb")
            nc.sync.dma_start(wd_sb[:], wd_src)

            if not nobias:
                sb_sb = constp.tile([128, NH * JC], F32, tag="sb_sb")
                nc.sync.dma_start(sb_sb[:], sbrep_d.to_broadcast((128, NH * JC)))
                bv8 = constp.tile([BL, HID], F32, tag="bv8")
                nc.sync.dma_start(bv8[:], bv_d.to_broadcast((BL, HID)))
                bd8 = constp.tile([BL, HID], F32, tag="bd8")
                nc.sync.dma_start(bd8[:], bd_d.to_broadcast((BL, HID)))
                cqn_bc = []
                for b in range(BL):
                    t = constp.tile([128, NH], F32, tag=f"cqn{b}", name=f"cqn{b}")
                    nc.sync.dma_start(
                        t[:], cqn_d[b : b + 1, :].to_broadcast((128, NH))
                    )
                    cqn_bc.append(t)
                ones16 = constp.tile([128, 1], BF16, tag="ones16")
                nc.vector.memset(ones16[:], 1.0)
                psp = pps.tile([128, BL], F32, tag="psp")

            # m output, already transposed: mT[c, ci, b*NH+h]
            mT = constp.tile([128, CI, BL * NH], BF16, tag="mT")

            # ---------------- main loop: per local batch ----------------
            for b in range(BL):
                ssq_all = gatep.tile([128, JC, NH], F32, tag="ssq", name="ssq_all")
                sk_all = gatep.tile([128, JC, SW], F32, tag="sk", name="sk_all")
                xn_tiles = []
                for jc in range(JC):
                    xnt = xnp.tile([128, HID], BF16, tag="xn", name="xnt")
                    nc.sync.dma_start(xnt[:], x[b, jc * 128 : (jc + 1) * 128, :])
                    xn_tiles.append(xnt)
                    # on-device transpose of the 8 subtiles -> x^T for kproj
                    # (all 8 land in one psum bank; one DVE eviction)
                    xtt = xtp.tile([128, CI, 128], BF16, tag="xt", name="xtt")
                    ptr = ptrp.tile([128, CI * 128], BF16, tag="ptr", name="ptr")
                    for cc in range(CI):
                        nc.tensor.transpose(
                            ptr[:, cc * 128 : (cc + 1) * 128],
                            xnt[:, cc * 128 : (cc + 1) * 128],
                            identB[:],
                        )
                    nc.vector.tensor_copy(
                        xtt[:], ptr[:].rearrange("p (ci j) -> p ci j", j=128)
                    )
                    # k-projection + score columns
                    pk = pp.tile([128, HID], F32, tag="pk", name="pk")
                    ps = pps.tile([128, SW], F32, tag="sml", name="ps")
                    for ci in range(CI):
                        st = ci == 0
                        sp_ = ci == CI - 1
                        lhs = xtt[:, ci, :]
                        for bank in range(2):
                            fs = slice(bank * 512, (bank + 1) * 512)
                            nc.tensor.matmul(
                                pk[:, fs], lhs, wk_sb[:, ci, fs], start=st, stop=sp_
                            )
                        nc.tensor.matmul(
                            ps[:, 0:SW], lhs, swblk[:, ci, b, :], start=st, stop=sp_
                        )
                    # evictions: k^2 via ACT square; segmented reduce on DVE
                    ksq = ksqp.tile([128, HID], F32, tag="ksq", name="ksq")
                    nc.scalar.activation(ksq[:], pk[:, :], AF.Square)
                    nc.vector.reduce_sum(
                        ssq_all[:, jc, :],
                        ksq[:].rearrange("p (h d) -> p h d", d=HD),
                        axis=AX.X,
                    )
                    nc.vector.tensor_copy(sk_all[:, jc, :], ps[:, 0:SW])

                # ---------------- gate (rational gumbel softmax) ----------
                # probs = p*A1/(p*A1 + (1-p)*A0) = p / (p + (1-p)*r), r=A0/A1
                rg_t = ap_pool.tile([128, JC, NH], F16, tag="rg", name="rg_t")
                nc.sync.dma_start(rg_t[:], rg[b])
                rgf = ap_pool.tile([128, JC, NH], F32, tag="rgf", name="rgf")
                nc.vector.tensor_copy(rgf[:], rg_t[:])

                g1 = gatep.tile([128, JC, NH], F32, tag="g1", name="g1")
                g2 = gatep.tile([128, JC, NH], F32, tag="g2", name="g2")
                g3 = gatep.tile([128, JC, NH], F32, tag="g3", name="g3")
                if nobias:
                    gden = ssq_all  # ||k||^2 needs no bias correction
                else:
                    nc.vector.scalar_tensor_tensor(
                        g1[:], sk_all[:, :, NH:SW], 2.0, ssq_all[:], OP.mult, OP.add
                    )
                    nc.vector.tensor_add(
                        g1[:], g1[:], sb_sb[:].rearrange("p (jc h) -> p jc h", h=NH)
                    )
                    gden = g1
                # g2 = rsqrt(gden) with one Newton step
                nc.scalar.activation(g2[:], gden[:], AF.Sqrt)
                nc.vector.reciprocal(g2[:], g2[:])
                nc.vector.tensor_mul(g3[:], g2[:], g2[:])
                nc.vector.tensor_mul(g3[:], g3[:], gden[:])
                nc.vector.tensor_scalar(g3[:], g3[:], -0.5, 1.5, OP.mult, OP.add)
                nc.vector.tensor_mul(g2[:], g2[:], g3[:])
                # g3 = scores = (s_raw (+ cqn)) * rsqrt
                if nobias:
                    nc.vector.tensor_mul(g3[:], sk_all[:, :, 0:NH], g2[:])
                else:
                    nc.vector.tensor_add(
                        g3[:],
                        sk_all[:, :, 0:NH],
                        cqn_bc[b][:].unsqueeze(1).to_broadcast([128, JC, NH]),
                    )
                    nc.vector.tensor_mul(g3[:], g3[:], g2[:])
                # p = (scores+1)/2 ; den = p + (1-p)*r
                nc.vector.tensor_scalar(g2[:], g3[:], 0.5, 0.5, OP.mult, OP.add)
                nc.vector.tensor_scalar(g1[:], g3[:], -0.5, 0.5, OP.mult, OP.add)
                nc.vector.tensor_mul(g1[:], g1[:], rgf[:])
                nc.vector.tensor_add(g1[:], g1[:], g2[:])  # den
                # probs = num * refined_recip(den)
                nc.vector.reciprocal(g3[:], g1[:])
                nc.vector.tensor_mul(g1[:], g1[:], g3[:])
                nc.vector.tensor_scalar(g1[:], g1[:], -1.0, 2.0, OP.mult, OP.add)
                nc.vector.tensor_mul(g3[:], g3[:], g1[:])
                nc.vector.tensor_mul(g2[:], g2[:], g3[:])  # probs (f32)
                probs = probsp.tile([128, JC, NH], BF16, tag="probs", name="probs")
                nc.vector.tensor_copy(probs[:], g2[:])

                # ---------------- m-matmul: m^T[c, h] = sum_j x[j,c] probs[j,h]
                # natural xn tiles as stationary -> output lands pre-transposed
                for cc in range(CI):
                    pm = pps.tile([128, NH], F32, tag="sml", name="pm")
                    for jc in range(JC):
                        nc.tensor.matmul(
                            pm[:, :],
                            xn_tiles[jc][:, cc * 128 : (cc + 1) * 128],
                            probs[:, jc, :],
                            start=(jc == 0),
                            stop=(jc == JC - 1),
                        )
                    nc.vector.tensor_copy(mT[:, cc, b * NH : (b + 1) * NH], pm[:, :])
                if not nobias:
                    # sp[h, b] = sum_j probs
                    for jc in range(JC):
                        nc.tensor.matmul(
                            psp[0:NH, b : b + 1],
                            probs[:, jc, :],
                            ones16[:],
                            start=(jc == 0),
                            stop=(jc == JC - 1),
                        )

            # ---------------- ctx + final dense ------------------------------
            # ctx[b, (h,d)] = sum_ci mT[:, ci, (b,h)] @ Wv[ci, (h,d)]
            pctx = pp.tile([128, HID], F32, tag="pk", name="pctx")
            for h in range(NH):
                for ci in range(CI):
                    nc.tensor.matmul(
                        pctx[0:BL, h * HD : (h + 1) * HD],
                        mT[:, ci, h : BL * NH : NH],
                        wv_sb[:, ci, h * HD : (h + 1) * HD],
                        start=(ci == 0),
                        stop=(ci == CI - 1),
                    )
            ctx_sb = constp.tile([BL, HID], BF16, tag="ctx_sb")
            if nobias:
                nc.vector.tensor_copy(ctx_sb[:], pctx[0:BL, :])
            else:
                # sp: psum [NH, BL] -> transpose -> [BL, NH]
                spT = constp.tile([NH, BL], BF16, tag="spT")
                nc.vector.tensor_copy(spT[:], psp[0:NH, 0:BL])
                ptr_sp = ptrp.tile([128, 128], BF16, tag="ptr", name="ptr_sp")
                nc.tensor.transpose(
                    ptr_sp[0:BL, 0:NH], spT[:], identB[0:NH, 0:NH]
                )
                sp_all = constp.tile([BL, NH], F32, tag="sp_all")
                nc.vector.tensor_copy(sp_all[:], ptr_sp[0:BL, 0:NH])
                ctxf = constp.tile([BL, HID], F32, tag="ctxf")
                nc.vector.tensor_mul(
                    ctxf[:].rearrange("b (h d) -> b h d", d=HD),
                    bv8[:].rearrange("b (h d) -> b h d", d=HD),
                    sp_all[:].unsqueeze(2).to_broadcast([BL, NH, HD]),
                )
                nc.vector.tensor_add(ctxf[:], ctxf[:], pctx[0:BL, :])
                nc.vector.tensor_copy(ctx_sb[:], ctxf[:])
            # transpose ctx -> [c, b]
            ctxT = constp.tile([128, CI, BL], BF16, tag="ctxT")
            for ci in range(CI):
                ptr_c = ptrp.tile([128, 128], BF16, tag="ptr", name="ptr_c")
                nc.tensor.transpose(
                    ptr_c[:, 0:BL],
                    ctx_sb[:, ci * 128 : (ci + 1) * 128],
                    identB[0:BL, 0:BL],
                )
                nc.vector.tensor_copy(ctxT[:, ci, :], ptr_c[:, 0:BL])
            # out = ctx @ Wd (+ bd)
            po = pp.tile([128, HID], F32, tag="pk", name="po")
            for ci in range(CI):
                st = ci == 0
                sp_ = ci == CI - 1
                for bank in range(2):
                    fs = slice(bank * 512, (bank + 1) * 512)
                    nc.tensor.matmul(
                        po[0:BL, fs], ctxT[:, ci, :], wd_sb[:, ci, fs],
                        start=st, stop=sp_,
                    )
            o_sb = constp.tile([BL, HID], F32, tag="o_sb")
            if nobias:
                nc.vector.tensor_copy(o_sb[:], po[0:BL, :])
            else:
                nc.vector.tensor_add(o_sb[:], po[0:BL, :], bd8[:])
            nc.sync.dma_start(out[:], o_sb[:])

    nc.compile()
    # the compiled module graph is large and permanent: freeze it out of
    # cyclic-GC scans so gen-2 collections during the steady-state dispatch
    # loop don't add tail latency
    import gc

    gc.collect()
    gc.freeze()
    return nc


def prep_in_maps(inputs, nobias=True, gather_w=True):
    """Host-side staging: batch shard, bf16/fp16 quantization, q-projection
    and gate-noise logs (both tiny compared to the device GEMMs)."""
    it = np.asarray(inputs["input_tensor"], dtype=np.float32)  # (B, 1, HID)
    rt = np.asarray(inputs["retrieval_tensor"], dtype=np.float32)  # (B, SK, HID)
    un = np.asarray(inputs["u_noise"], dtype=np.float32)  # (B, NH, 1, SK, 2)
    Wq = np.asarray(inputs["Wq"], dtype=np.float32)
    Wk = np.asarray(inputs["Wk"], dtype=np.float32)
    Wv = np.asarray(inputs["Wv"], dtype=np.float32)
    Wd = np.asarray(inputs["Wd"], dtype=np.float32)
    bq = np.asarray(inputs["bq"], dtype=np.float32).reshape(HID)

    # qhat on host (f32, matches reference precision); bq folded in here
    q = it[:, 0, :] @ Wq + bq
    qh = q.reshape(B, NH, HD)
    qh = qh / np.linalg.norm(qh, axis=-1, keepdims=True)  # (B, NH, HD)

    # swblk[c, (b,h)] = Wk @ qhat_blockdiag, laid out [p, ci, b, h]; bias
    # builds append the wbk norm-correction columns (same for every b)
    Wkh = Wk.reshape(HID, NH, HD)
    wq_eff = np.einsum("chd,bhd->cbh", Wkh, qh, optimize=True)  # (HID, B, NH)
    if not nobias:
        bk = np.asarray(inputs["bk"], dtype=np.float32).reshape(NH, HD)
        wbk = np.einsum("chd,hd->ch", Wkh, bk)  # (HID, NH)
        wq_eff = np.concatenate(
            [wq_eff, np.broadcast_to(wbk[:, None, :], wq_eff.shape)], axis=2
        )  # (HID, B, 2*NH)
    sw = wq_eff.shape[2]
    swblk = np.ascontiguousarray(
        wq_eff.reshape(CI, 128, B, sw).transpose(1, 0, 2, 3)
    ).astype(ml_dtypes.bfloat16)  # (128, CI, B, SW)

    # gate noise ratio r = A0/A1, A_i = EPS - log(u_i + EPS). Only the ratio
    # enters probs = p/(p + (1-p)r), and its fp16 relative error stays bounded
    # (Delta probs = probs(1-probs) * dr/r). Clamped into fp16's finite
    # normal range; both saturation limits give the correct probs limit.
    u0 = un[:, :, 0, :, 0].transpose(0, 2, 1)  # (B, SK, NH)
    u1 = un[:, :, 0, :, 1].transpose(0, 2, 1)
    a0 = np.float32(EPS) - np.log(u0 + np.float32(EPS), dtype=np.float32)
    a1 = np.float32(EPS) - np.log(u1 + np.float32(EPS), dtype=np.float32)
    rg = np.clip(a0 / a1, np.float32(6.2e-5), np.float32(60000.0)).astype(
        np.float16
    )
    # pre-tile to [b, j%128, jc, h] for contiguous DMA partition lines
    rg = np.ascontiguousarray(rg.reshape(B, JC, 128, NH).transpose(0, 2, 1, 3))

    x_bf = rt.astype(ml_dtypes.bfloat16)  # (B, SK, HID)

    # stacked [Wk, Wv, Wd] for the per-core weight shards
    wkvd = np.stack([Wk, Wv, Wd]).astype(ml_dtypes.bfloat16)  # (3, HID, HID)

    shared = {}
    if not gather_w:
        shared.update(wk=wkvd[0], wv=wkvd[1], wd=wkvd[2])
    if not nobias:
        shared["sbrep"] = (
            np.tile((bk**2).sum(axis=1), JC).reshape(1, NH * JC).astype(np.float32)
        )
        shared["bv"] = np.asarray(inputs["bv"], dtype=np.float32).reshape(1, HID)
        shared["bd"] = np.asarray(inputs["bd"], dtype=np.float32).reshape(1, HID)
        cqn = np.einsum("bhd,hd->bh", qh, bk).astype(np.float32)  # (B, NH)

    in_maps = []
    for c in range(NCORES):
        bs = slice(c * BL, (c + 1) * BL)
        m = {
            "x": np.ascontiguousarray(x_bf[bs]),
            "rg": np.ascontiguousarray(rg[bs]),
            "swblk": np.ascontiguousarray(swblk[:, :, bs, :]),
            **shared,
        }
        if gather_w:
            m["wshard"] = np.ascontiguousarray(wkvd[:, c * 128 : (c + 1) * 128, :])
        if not nobias:
            m["cqn"] = np.ascontiguousarray(cqn[bs])
        in_maps.append(m)
    return in_maps


_NC_CACHE = {}


def kernel(**inputs) -> np.ndarray:
    # bq is folded into the host-side q-projection, so only bk/bv/bd need the
    # device bias path
    nobias = all(
        not np.any(np.asarray(inputs[k])) for k in ("bk", "bv", "bd")
    )
    if nobias not in _NC_CACHE:
        _NC_CACHE[nobias] = build_nc(nobias)
    nc = _NC_CACHE[nobias]
    in_maps = prep_in_maps(inputs, nobias)
    res = run_bass_kernel_spmd(nc, in_maps, core_ids=list(range(NCORES)))
    return np.concatenate([res.results[c]["o"] for c in range(NCORES)], axis=0)
